# revision 17
# baseline (speedup 1.0000x reference)
"""NeuMF (embedding lookup + tiny MLP) on 8 Trainium2 NeuronCores.

Strategy (data-parallel: replicate tables, shard the 16384 ids 8 ways):
- Host: build one combined bf16 table cucm[(NU+NM), 72]; user row r =
  [gmf_user[r] | mlp_user[r]], movie rows offset by NU with Wf[0:64]
  pre-folded into the gmf columns (so the GMF logit is a plain
  mul+reduce on chip).
- Each core gathers its 2048 user + 2048 movie rows with 32 indirect
  DMAs (walrus semantics: one dynamic offset per partition, so 128
  rows per instruction is the max).  The ~1us/instruction SWDGE cost
  dominates the kernel; everything else is pipelined under it:
    prod  = gmf_u * gmf_m'            (DVE, bf16 2x, per 2-t-block slice)
    pgc   = reduce_e prod             (DVE)   -> gmf logit [p, t]
    mlpc  = packed mlp halves         (DVE copies)
    mT    = PE-transpose of mlpc      (per chunk), PSUM->SBUF on ACT
    h1    = relu(W1bd^T mT + b1)      (PE + ACT, block-diag over t)
    h2    = relu(W2bd^T h1 + b2)
    logit = Wf4bd^T h2 (+) pgc^T      (PSUM-accumulated)
    out   = 4*sigmoid(logit + bf) + 1 (ACT sigmoid + DVE affine)
  MLP chunks are sized [8t, 6t, 2t] so the last chunk's work after the
  final gather lands is minimal.
- Gather/unshard on host (pure layout permutation, no embedding math).
"""
import sys
import types
import functools

import numpy as np

# ---------------- problem constants (hardcoded per contract) ----------------
NU = 1_000_000
NM = 100_000
E = 64            # gmf embed dim
MD = 8            # mlp half dim
CW = E + MD       # combined row width (72)
B = 16384
NCORES = 8
SHARD = B // NCORES   # 2048
P = 128
T = SHARD // P        # 16 t-blocks per core
MLP_CHUNKS = ((0, 8), (8, 6), (14, 2))   # (t0, nt) per MLP chunk

TRACE = False          # test.py flips this for neuron-profile timing
LAST_EXEC_NS = None


def _install_ntff_hook():
    """bass_utils' trace path imports antenv.axon_hooks (absent here); shim it."""
    if "antenv.axon_hooks" in sys.modules:
        return
    try:
        import antenv  # noqa: F401
        mod = types.ModuleType("antenv.axon_hooks")
        mod._hook = None
        mod.set_axon_ntff_profile_hook = lambda h: setattr(mod, "_hook", h)
        mod.get_axon_ntff_profile_hook = lambda: mod._hook
        sys.modules["antenv.axon_hooks"] = mod
        from trn_agent_boot.trn_boot import _ntff_profile_via_ctypes
        mod.set_axon_ntff_profile_hook(
            _ntff_profile_via_ctypes('/opt/axon/libaxon_pjrt.so'))
    except Exception:
        pass


@functools.lru_cache(maxsize=1)
def _build_program():
    import concourse.bacc as bacc
    import concourse.bass as bass
    import concourse.tile as tile
    from concourse import mybir
    from concourse.mybir import ActivationFunctionType as AFT

    f32 = mybir.dt.float32
    bf16 = mybir.dt.bfloat16
    i32 = mybir.dt.int32

    nc = bacc.Bacc("TRN2", target_bir_lowering=False, debug=False,
                   enable_asserts=False, num_devices=NCORES)

    # ids: [128, 32] int32; col 2t = user idx of t-block t, col 2t+1 = movie+NU
    ids_d = nc.dram_tensor("ids", (P, 2 * T), i32, kind="ExternalInput")
    tab_d = nc.dram_tensor("tab", (NU + NM, CW), bf16, kind="ExternalInput")
    # cb: bf16 consts [128, 272] = [identity | W1bd | W2bd(r0:64) | Wf4s(r0:32)]
    cb_d = nc.dram_tensor("cb", (P, 272), bf16, kind="ExternalInput")
    # cf: f32 consts [128, 131] = [identity_f32 | b1r | b2r | bfr]
    cf_d = nc.dram_tensor("cf", (P, 131), f32, kind="ExternalInput")
    out_d = nc.dram_tensor("out", (SHARD,), f32, kind="ExternalOutput")

    with tile.TileContext(nc) as tc:
        with (
            tc.tile_pool(name="const", bufs=1) as cpool,
            tc.tile_pool(name="gat", bufs=1) as gpool,
            tc.tile_pool(name="work", bufs=2) as wpool,
            tc.tile_pool(name="ps_t", bufs=2, space="PSUM") as pt_pool,
            tc.tile_pool(name="ps_m", bufs=2, space="PSUM") as pm_pool,
            tc.tile_pool(name="ps_l", bufs=1, space="PSUM") as pl_pool,
        ):
            # ids via the scalar engine: with no early ACT work, its
            # first DMA slot comes before sync's (entry drain is shorter).
            ids = cpool.tile([P, 2 * T], i32)
            nc.scalar.dma_start(out=ids[:], in_=ids_d[:])
            cb = cpool.tile([P, 272], bf16)
            nc.sync.dma_start(out=cb[:], in_=cb_d[:])
            cf = cpool.tile([P, 131], f32)
            nc.sync.dma_start(out=cf[:], in_=cf_d[:])

            idb = cb[:, 0:128]            # bf16 identity
            w1bd = cb[:, 128:192]         # [128, 64]
            w2bd = cb[0:64, 192:224]      # [64, 32]
            wf4s = cb[0:32, 224:248]      # [32, 3 chunks x 8]
            idf = cf[:, 0:128]            # f32 identity
            b1r = cf[0:64, 128:129]
            b2r = cf[0:32, 129:130]
            bfr = cf[0:16, 130:131]

            # ---- gather: 32 indirect DMAs, one [128, 72] row-block each ----
            g = gpool.tile([P, 2 * T * CW], bf16)   # [128, 32, 72] flat
            for c in range(2 * T):
                nc.gpsimd.indirect_dma_start(
                    out=g[:, c * CW:(c + 1) * CW],
                    out_offset=None,
                    in_=tab_d[:],
                    in_offset=bass.IndirectOffsetOnAxis(ap=ids[:, c:c + 1],
                                                        axis=0),
                )

            g3 = g[:].rearrange("p (c w) -> p c w", w=CW)   # [128, 32, 72]
            pgc = wpool.tile([P, T], f32, bufs=1)   # gmf logits, [p, t]
            mlpc = wpool.tile([P, T * 2 * MD], bf16, bufs=1)
            m3 = mlpc[:].rearrange("p (t k) -> p t k", k=2 * MD)

            # GMF mul+reduce slices and per-chunk MLP packs, emitted in data-
            # arrival order so no DVE op queues behind a later-satisfied wait.
            # The gather column order (u,m,u,m,...) already matches the
            # [user|movie] k-layout, so each pack is ONE strided copy.
            def pack(t0, nt):
                nc.vector.tensor_copy(out=m3[:, t0:t0 + nt, :],
                                      in_=g3[:, 2 * t0:2 * (t0 + nt), E:CW])

            def gmf(s0, sn):
                prod = wpool.tile([P, 4 * E], bf16, name="prod")
                p3 = prod[:].rearrange("p (t e) -> p t e", e=E)[:, 0:sn, :]
                nc.vector.tensor_mul(out=p3,
                                     in0=g3[:, 2 * s0:2 * (s0 + sn):2, 0:E],
                                     in1=g3[:, 2 * s0 + 1:2 * (s0 + sn):2, 0:E])
                nc.vector.tensor_reduce(out=pgc[:, s0:s0 + sn], in_=p3,
                                        axis=mybir.AxisListType.X,
                                        op=mybir.AluOpType.add)

            gmf(0, 4)            # needs DMAs 1..8
            pack(0, 8)           # needs DMAs 1..16 (unblocks chunk-A MLP)
            gmf(4, 4)            # needs DMAs 9..16
            gmf(8, 4)            # needs DMAs 17..24
            pack(8, 6)           # needs DMAs 17..28 (unblocks chunk-B MLP)
            gmf(12, 2)           # needs DMAs 25..28
            pack(14, 2)          # needs DMAs 29..32 (unblocks chunk-C MLP)
            gmf(14, 2)           # needs DMAs 29..32

            # MLP chunks (block-diag weights cover nt t-blocks at a time);
            # each chunk owns its logit PSUM region and stores its rows as
            # soon as they are done, so only the last (2-t) chunk sits on
            # the critical tail.
            od = out_d[:].rearrange("(t p) -> t p", p=P)
            for ci, (t0, nt) in enumerate(MLP_CHUNKS):
                kw = nt * 2 * MD          # mT partition count
                mT_ps = pt_pool.tile([P, P], bf16, space="PSUM", name="mT_ps")
                nc.tensor.transpose(out=mT_ps[0:kw, :],
                                    in_=mlpc[:, t0 * 2 * MD:(t0 + nt) * 2 * MD],
                                    identity=idb)
                mT = wpool.tile([P, P], bf16, name="mT")
                nc.scalar.activation(out=mT[0:kw, :], in_=mT_ps[0:kw, :],
                                     func=AFT.Copy)
                h1_ps = pm_pool.tile([64, P], f32, space="PSUM", name="h1_ps")
                nc.tensor.matmul(out=h1_ps[0:8 * nt, :],
                                 lhsT=w1bd[0:kw, 0:8 * nt], rhs=mT[0:kw, :],
                                 start=True, stop=True)
                h1 = wpool.tile([64, P], bf16, name="h1")
                nc.scalar.activation(out=h1[0:8 * nt, :], in_=h1_ps[0:8 * nt, :],
                                     func=AFT.Relu, bias=b1r[0:8 * nt, :])
                h2_ps = pm_pool.tile([32, P], f32, space="PSUM", name="h2_ps")
                nc.tensor.matmul(out=h2_ps[0:4 * nt, :],
                                 lhsT=w2bd[0:8 * nt, 0:4 * nt], rhs=h1[0:8 * nt, :],
                                 start=True, stop=True)
                h2 = wpool.tile([32, P], bf16, name="h2")
                nc.scalar.activation(out=h2[0:4 * nt, :], in_=h2_ps[0:4 * nt, :],
                                     func=AFT.Relu, bias=b2r[0:4 * nt, :])
                logit = pl_pool.tile([8, P], f32, space="PSUM", name="logit")
                # gmf logits for this chunk's rows: logit = pgc[:, t0:t0+nt]^T
                nc.tensor.matmul(out=logit[0:nt, :],
                                 lhsT=pgc[:, t0:t0 + nt], rhs=idf,
                                 is_transpose=True, start=True, stop=False,
                                 skip_group_check=True)
                # wf4s chunk slice: [4*nt, nt] -> chunk-local logit rows
                nc.tensor.matmul(out=logit[0:nt, :],
                                 lhsT=wf4s[0:4 * nt, 8 * ci:8 * ci + nt],
                                 rhs=h2[0:4 * nt, :],
                                 start=False, stop=True,
                                 skip_group_check=True)
                sg = wpool.tile([8, P], f32, name="sg")
                nc.scalar.activation(out=sg[0:nt, :], in_=logit[0:nt, :],
                                     func=AFT.Sigmoid, bias=bfr[0:nt, :])
                o = wpool.tile([8, P], f32, name="o")
                nc.vector.tensor_scalar(out=o[0:nt, :], in0=sg[0:nt, :],
                                        scalar1=4.0, scalar2=1.0,
                                        op0=mybir.AluOpType.mult,
                                        op1=mybir.AluOpType.add)
                nc.sync.dma_start(out=od[t0:t0 + nt, :], in_=o[0:nt, :])

    nc.compile()
    return nc


def _host_prep(user_ids, movie_ids, gmf_user_emb, gmf_movie_emb,
               mlp_user_emb, mlp_movie_emb, W1, b1, W2, b2, Wf, bf):
    """Build the combined bf16 table, per-core ids, and constant tensors."""
    from concourse import mybir
    npbf = mybir.dt.np(mybir.dt.bfloat16)

    uid = np.asarray(user_ids).astype(np.int32)
    mid = np.asarray(movie_ids).astype(np.int32)
    W1 = np.asarray(W1, np.float32)
    W2 = np.asarray(W2, np.float32)
    Wf = np.asarray(Wf, np.float32)
    b1 = np.asarray(b1, np.float32)
    b2 = np.asarray(b2, np.float32)
    bfv = float(np.asarray(bf).reshape(-1)[0])

    tab = np.empty((NU + NM, CW), npbf)
    tab[:NU, :E] = np.asarray(gmf_user_emb, np.float32).astype(npbf)
    tab[:NU, E:] = np.asarray(mlp_user_emb, np.float32).astype(npbf)
    # fold Wf[0:64] into the movie gmf columns (f32 product, then bf16)
    tab[NU:, :E] = (np.asarray(gmf_movie_emb, np.float32)
                    * Wf[0:E, 0][None, :]).astype(npbf)
    tab[NU:, E:] = np.asarray(mlp_movie_emb, np.float32).astype(npbf)

    # W1 blockdiag over 8 t_l blocks: [128=(t_l,k), 64=(t_l,j)]
    w1bd = np.zeros((P, 64), np.float32)
    for tl in range(8):
        w1bd[tl * 16:(tl + 1) * 16, tl * 8:(tl + 1) * 8] = W1
    # W2 blockdiag: [64=(t_l,j), 32=(t_l,l)]
    w2bd = np.zeros((64, 32), np.float32)
    for tl in range(8):
        w2bd[tl * 8:(tl + 1) * 8, tl * 4:(tl + 1) * 4] = W2
    # wf4 stage per MLP chunk ci: lhsT slice [0:4*nt, 8*ci:8*ci+nt];
    # within the slice, column tl (the chunk-local logit row) gets
    # Wf[64:68] at rows tl*4:(tl+1)*4.
    wf4s = np.zeros((32, 24), np.float32)
    for ci, (t0, nt) in enumerate(MLP_CHUNKS):
        for tl in range(nt):
            wf4s[tl * 4:(tl + 1) * 4, 8 * ci + tl] = Wf[E:E + 4, 0]

    cb = np.zeros((P, 272), np.float32)
    cb[:, 0:128] = np.eye(P, dtype=np.float32)
    cb[:, 128:192] = w1bd
    cb[0:64, 192:224] = w2bd
    cb[0:32, 224:248] = wf4s
    cb = cb.astype(npbf)

    cf = np.zeros((P, 131), np.float32)
    cf[:, 0:128] = np.eye(P, dtype=np.float32)
    cf[0:64, 128:129] = np.tile(b1, 8).reshape(64, 1)
    cf[0:32, 129:130] = np.tile(b2, 8).reshape(32, 1)
    cf[0:16, 130:131] = bfv

    in_maps = []
    for c in range(NCORES):
        us = uid[c * SHARD:(c + 1) * SHARD]
        ms = mid[c * SHARD:(c + 1) * SHARD] + NU
        # batch b = t*128 + p maps to ids[p, 2t] / ids[p, 2t+1]
        ids = np.empty((P, 2 * T), np.int32)
        ids[:, 0::2] = us.reshape(T, P).T
        ids[:, 1::2] = ms.reshape(T, P).T
        in_maps.append({"ids": ids, "tab": tab, "cb": cb, "cf": cf})
    return in_maps


def kernel(**inputs) -> np.ndarray:
    global LAST_EXEC_NS
    _install_ntff_hook()
    from concourse.bass_utils import run_bass_kernel_spmd

    nc = _build_program()
    in_maps = _host_prep(**inputs)
    res = run_bass_kernel_spmd(nc, in_maps, list(range(NCORES)), trace=TRACE)
    LAST_EXEC_NS = res.exec_time_ns
    out = np.concatenate([res.results[c]["out"] for c in range(NCORES)])
    return out.astype(np.float32)


# revision 18
# speedup vs baseline: 1.0028x; 1.0028x over previous
"""NeuMF (embedding lookup + tiny MLP) on 8 Trainium2 NeuronCores.

Strategy (data-parallel: replicate tables, shard the 16384 ids 8 ways):
- Host: build one combined bf16 table cucm[(NU+NM), 72]; user row r =
  [gmf_user[r] | mlp_user[r]], movie rows offset by NU with Wf[0:64]
  pre-folded into the gmf columns (so the GMF logit is a plain
  mul+reduce on chip).
- Each core gathers its 2048 user + 2048 movie rows with 32 indirect
  DMAs (walrus semantics: one dynamic offset per partition, so 128
  rows per instruction is the max).  The ~1us/instruction SWDGE cost
  dominates the kernel; everything else is pipelined under it:
    prod  = gmf_u * gmf_m'            (DVE, bf16 2x, per 2-t-block slice)
    pgc   = reduce_e prod             (DVE)   -> gmf logit [p, t]
    mlpc  = packed mlp halves         (DVE copies)
    mT    = PE-transpose of mlpc      (per chunk), PSUM->SBUF on ACT
    h1    = relu(W1bd^T mT + b1)      (PE + ACT, block-diag over t)
    h2    = relu(W2bd^T h1 + b2)
    logit = Wf4bd^T h2 (+) pgc^T      (PSUM-accumulated)
    out   = 4*sigmoid(logit + bf) + 1 (ACT sigmoid + DVE affine)
  MLP chunks are sized [8t, 6t, 2t] so the last chunk's work after the
  final gather lands is minimal.
- Gather/unshard on host (pure layout permutation, no embedding math).
"""
import sys
import types
import functools

import numpy as np

# ---------------- problem constants (hardcoded per contract) ----------------
NU = 1_000_000
NM = 100_000
E = 64            # gmf embed dim
MD = 8            # mlp half dim
CW = E + MD       # combined row width (72)
B = 16384
NCORES = 8
SHARD = B // NCORES   # 2048
P = 128
T = SHARD // P        # 16 t-blocks per core
MLP_CHUNKS = ((0, 8), (8, 6), (14, 2))   # (t0, nt) per MLP chunk

TRACE = False          # test.py flips this for neuron-profile timing
LAST_EXEC_NS = None


def _install_ntff_hook():
    """bass_utils' trace path imports antenv.axon_hooks (absent here); shim it."""
    if "antenv.axon_hooks" in sys.modules:
        return
    try:
        import antenv  # noqa: F401
        mod = types.ModuleType("antenv.axon_hooks")
        mod._hook = None
        mod.set_axon_ntff_profile_hook = lambda h: setattr(mod, "_hook", h)
        mod.get_axon_ntff_profile_hook = lambda: mod._hook
        sys.modules["antenv.axon_hooks"] = mod
        from trn_agent_boot.trn_boot import _ntff_profile_via_ctypes
        mod.set_axon_ntff_profile_hook(
            _ntff_profile_via_ctypes('/opt/axon/libaxon_pjrt.so'))
    except Exception:
        pass


@functools.lru_cache(maxsize=1)
def _build_program():
    import concourse.bacc as bacc
    import concourse.bass as bass
    import concourse.tile as tile
    from concourse import mybir
    from concourse.mybir import ActivationFunctionType as AFT

    f32 = mybir.dt.float32
    bf16 = mybir.dt.bfloat16
    i32 = mybir.dt.int32

    nc = bacc.Bacc("TRN2", target_bir_lowering=False, debug=False,
                   enable_asserts=False, num_devices=NCORES)

    # ids: [128, 32] int32; col 2t = user idx of t-block t, col 2t+1 = movie+NU
    ids_d = nc.dram_tensor("ids", (P, 2 * T), i32, kind="ExternalInput")
    tab_d = nc.dram_tensor("tab", (NU + NM, CW), bf16, kind="ExternalInput")
    # cb: bf16 consts [128, 272] = [identity | W1bd | W2bd(r0:64) | Wf4s(r0:32)]
    cb_d = nc.dram_tensor("cb", (P, 272), bf16, kind="ExternalInput")
    # cf: f32 consts [128, 131] = [identity_f32 | b1r | b2r | bfr]
    cf_d = nc.dram_tensor("cf", (P, 131), f32, kind="ExternalInput")
    out_d = nc.dram_tensor("out", (SHARD,), f32, kind="ExternalOutput")

    with tile.TileContext(nc) as tc:
        with (
            tc.tile_pool(name="const", bufs=1) as cpool,
            tc.tile_pool(name="gat", bufs=1) as gpool,
            tc.tile_pool(name="work", bufs=2) as wpool,
            tc.tile_pool(name="ps_t", bufs=2, space="PSUM") as pt_pool,
            tc.tile_pool(name="ps_m", bufs=2, space="PSUM") as pm_pool,
            tc.tile_pool(name="ps_l", bufs=1, space="PSUM") as pl_pool,
        ):
            ids = cpool.tile([P, 2 * T], i32)
            nc.sync.dma_start(out=ids[:], in_=ids_d[:])
            cb = cpool.tile([P, 272], bf16)
            nc.sync.dma_start(out=cb[:], in_=cb_d[:])
            cf = cpool.tile([P, 131], f32)
            nc.sync.dma_start(out=cf[:], in_=cf_d[:])

            idb = cb[:, 0:128]            # bf16 identity
            w1bd = cb[:, 128:192]         # [128, 64]
            w2bd = cb[0:64, 192:224]      # [64, 32]
            wf4s = cb[0:32, 224:248]      # [32, 3 chunks x 8]
            idf = cf[:, 0:128]            # f32 identity
            b1r = cf[0:64, 128:129]
            b2r = cf[0:32, 129:130]
            bfr = cf[0:16, 130:131]

            # warm the sigmoid ACT LUT during the gathers, off the critical path
            warm = wpool.tile([1, 1], f32, bufs=1)
            nc.scalar.activation(out=warm[:], in_=cf[0:1, 130:131],
                                 func=AFT.Sigmoid, bias=bfr[0:1, :])

            # ---- gather: 32 indirect DMAs, one [128, 72] row-block each ----
            g = gpool.tile([P, 2 * T * CW], bf16)   # [128, 32, 72] flat
            for c in range(2 * T):
                nc.gpsimd.indirect_dma_start(
                    out=g[:, c * CW:(c + 1) * CW],
                    out_offset=None,
                    in_=tab_d[:],
                    in_offset=bass.IndirectOffsetOnAxis(ap=ids[:, c:c + 1],
                                                        axis=0),
                )

            g3 = g[:].rearrange("p (c w) -> p c w", w=CW)   # [128, 32, 72]
            pgc = wpool.tile([P, T], f32, bufs=1)   # gmf logits, [p, t]
            mlpc = wpool.tile([P, T * 2 * MD], bf16, bufs=1)
            m3 = mlpc[:].rearrange("p (t k) -> p t k", k=2 * MD)

            # GMF mul+reduce slices and per-chunk MLP packs, emitted in data-
            # arrival order so no DVE op queues behind a later-satisfied wait.
            # The gather column order (u,m,u,m,...) already matches the
            # [user|movie] k-layout, so each pack is ONE strided copy.
            def pack(t0, nt):
                nc.vector.tensor_copy(out=m3[:, t0:t0 + nt, :],
                                      in_=g3[:, 2 * t0:2 * (t0 + nt), E:CW])

            def gmf(s0, sn):
                prod = wpool.tile([P, 4 * E], bf16, name="prod")
                p3 = prod[:].rearrange("p (t e) -> p t e", e=E)[:, 0:sn, :]
                nc.vector.tensor_mul(out=p3,
                                     in0=g3[:, 2 * s0:2 * (s0 + sn):2, 0:E],
                                     in1=g3[:, 2 * s0 + 1:2 * (s0 + sn):2, 0:E])
                nc.vector.tensor_reduce(out=pgc[:, s0:s0 + sn], in_=p3,
                                        axis=mybir.AxisListType.X,
                                        op=mybir.AluOpType.add)

            gmf(0, 4)            # needs DMAs 1..8
            pack(0, 8)           # needs DMAs 1..16 (unblocks chunk-A MLP)
            gmf(4, 4)            # needs DMAs 9..16
            gmf(8, 4)            # needs DMAs 17..24
            pack(8, 6)           # needs DMAs 17..28 (unblocks chunk-B MLP)
            gmf(12, 2)           # needs DMAs 25..28
            pack(14, 2)          # needs DMAs 29..32 (unblocks chunk-C MLP)
            gmf(14, 2)           # needs DMAs 29..32

            # MLP chunks (block-diag weights cover nt t-blocks at a time);
            # each chunk owns its logit PSUM region and stores its rows as
            # soon as they are done, so only the last (2-t) chunk sits on
            # the critical tail.
            od = out_d[:].rearrange("(t p) -> t p", p=P)
            for ci, (t0, nt) in enumerate(MLP_CHUNKS):
                kw = nt * 2 * MD          # mT partition count
                mT_ps = pt_pool.tile([P, P], bf16, space="PSUM", name="mT_ps")
                nc.tensor.transpose(out=mT_ps[0:kw, :],
                                    in_=mlpc[:, t0 * 2 * MD:(t0 + nt) * 2 * MD],
                                    identity=idb)
                mT = wpool.tile([P, P], bf16, name="mT")
                nc.scalar.activation(out=mT[0:kw, :], in_=mT_ps[0:kw, :],
                                     func=AFT.Copy)
                h1_ps = pm_pool.tile([64, P], f32, space="PSUM", name="h1_ps")
                nc.tensor.matmul(out=h1_ps[0:8 * nt, :],
                                 lhsT=w1bd[0:kw, 0:8 * nt], rhs=mT[0:kw, :],
                                 start=True, stop=True)
                h1 = wpool.tile([64, P], bf16, name="h1")
                nc.scalar.activation(out=h1[0:8 * nt, :], in_=h1_ps[0:8 * nt, :],
                                     func=AFT.Relu, bias=b1r[0:8 * nt, :])
                h2_ps = pm_pool.tile([32, P], f32, space="PSUM", name="h2_ps")
                nc.tensor.matmul(out=h2_ps[0:4 * nt, :],
                                 lhsT=w2bd[0:8 * nt, 0:4 * nt], rhs=h1[0:8 * nt, :],
                                 start=True, stop=True)
                h2 = wpool.tile([32, P], bf16, name="h2")
                nc.scalar.activation(out=h2[0:4 * nt, :], in_=h2_ps[0:4 * nt, :],
                                     func=AFT.Relu, bias=b2r[0:4 * nt, :])
                logit = pl_pool.tile([8, P], f32, space="PSUM", name="logit")
                # gmf logits for this chunk's rows: logit = pgc[:, t0:t0+nt]^T
                nc.tensor.matmul(out=logit[0:nt, :],
                                 lhsT=pgc[:, t0:t0 + nt], rhs=idf,
                                 is_transpose=True, start=True, stop=False,
                                 skip_group_check=True)
                # wf4s chunk slice: [4*nt, nt] -> chunk-local logit rows
                nc.tensor.matmul(out=logit[0:nt, :],
                                 lhsT=wf4s[0:4 * nt, 8 * ci:8 * ci + nt],
                                 rhs=h2[0:4 * nt, :],
                                 start=False, stop=True,
                                 skip_group_check=True)
                sg = wpool.tile([8, P], f32, name="sg")
                nc.scalar.activation(out=sg[0:nt, :], in_=logit[0:nt, :],
                                     func=AFT.Sigmoid, bias=bfr[0:nt, :])
                o = wpool.tile([8, P], f32, name="o")
                nc.vector.tensor_scalar(out=o[0:nt, :], in0=sg[0:nt, :],
                                        scalar1=4.0, scalar2=1.0,
                                        op0=mybir.AluOpType.mult,
                                        op1=mybir.AluOpType.add)
                nc.sync.dma_start(out=od[t0:t0 + nt, :], in_=o[0:nt, :])

    nc.compile()
    return nc


def _host_prep(user_ids, movie_ids, gmf_user_emb, gmf_movie_emb,
               mlp_user_emb, mlp_movie_emb, W1, b1, W2, b2, Wf, bf):
    """Build the combined bf16 table, per-core ids, and constant tensors."""
    from concourse import mybir
    npbf = mybir.dt.np(mybir.dt.bfloat16)

    uid = np.asarray(user_ids).astype(np.int32)
    mid = np.asarray(movie_ids).astype(np.int32)
    W1 = np.asarray(W1, np.float32)
    W2 = np.asarray(W2, np.float32)
    Wf = np.asarray(Wf, np.float32)
    b1 = np.asarray(b1, np.float32)
    b2 = np.asarray(b2, np.float32)
    bfv = float(np.asarray(bf).reshape(-1)[0])

    tab = np.empty((NU + NM, CW), npbf)
    tab[:NU, :E] = np.asarray(gmf_user_emb, np.float32).astype(npbf)
    tab[:NU, E:] = np.asarray(mlp_user_emb, np.float32).astype(npbf)
    # fold Wf[0:64] into the movie gmf columns (f32 product, then bf16)
    tab[NU:, :E] = (np.asarray(gmf_movie_emb, np.float32)
                    * Wf[0:E, 0][None, :]).astype(npbf)
    tab[NU:, E:] = np.asarray(mlp_movie_emb, np.float32).astype(npbf)

    # W1 blockdiag over 8 t_l blocks: [128=(t_l,k), 64=(t_l,j)]
    w1bd = np.zeros((P, 64), np.float32)
    for tl in range(8):
        w1bd[tl * 16:(tl + 1) * 16, tl * 8:(tl + 1) * 8] = W1
    # W2 blockdiag: [64=(t_l,j), 32=(t_l,l)]
    w2bd = np.zeros((64, 32), np.float32)
    for tl in range(8):
        w2bd[tl * 8:(tl + 1) * 8, tl * 4:(tl + 1) * 4] = W2
    # wf4 stage per MLP chunk ci: lhsT slice [0:4*nt, 8*ci:8*ci+nt];
    # within the slice, column tl (the chunk-local logit row) gets
    # Wf[64:68] at rows tl*4:(tl+1)*4.
    wf4s = np.zeros((32, 24), np.float32)
    for ci, (t0, nt) in enumerate(MLP_CHUNKS):
        for tl in range(nt):
            wf4s[tl * 4:(tl + 1) * 4, 8 * ci + tl] = Wf[E:E + 4, 0]

    cb = np.zeros((P, 272), np.float32)
    cb[:, 0:128] = np.eye(P, dtype=np.float32)
    cb[:, 128:192] = w1bd
    cb[0:64, 192:224] = w2bd
    cb[0:32, 224:248] = wf4s
    cb = cb.astype(npbf)

    cf = np.zeros((P, 131), np.float32)
    cf[:, 0:128] = np.eye(P, dtype=np.float32)
    cf[0:64, 128:129] = np.tile(b1, 8).reshape(64, 1)
    cf[0:32, 129:130] = np.tile(b2, 8).reshape(32, 1)
    cf[0:16, 130:131] = bfv

    in_maps = []
    for c in range(NCORES):
        us = uid[c * SHARD:(c + 1) * SHARD]
        ms = mid[c * SHARD:(c + 1) * SHARD] + NU
        # batch b = t*128 + p maps to ids[p, 2t] / ids[p, 2t+1]
        ids = np.empty((P, 2 * T), np.int32)
        ids[:, 0::2] = us.reshape(T, P).T
        ids[:, 1::2] = ms.reshape(T, P).T
        in_maps.append({"ids": ids, "tab": tab, "cb": cb, "cf": cf})
    return in_maps


def kernel(**inputs) -> np.ndarray:
    global LAST_EXEC_NS
    _install_ntff_hook()
    from concourse.bass_utils import run_bass_kernel_spmd

    nc = _build_program()
    in_maps = _host_prep(**inputs)
    res = run_bass_kernel_spmd(nc, in_maps, list(range(NCORES)), trace=TRACE)
    LAST_EXEC_NS = res.exec_time_ns
    out = np.concatenate([res.results[c]["out"] for c in range(NCORES)])
    return out.astype(np.float32)
